# revision 10
# baseline (speedup 1.0000x reference)
"""ChildSum TreeLSTM on 8 Trainium2 NeuronCores (Bass/Tile).

Strategy:
  - Host: compute node levels; partition the bottom of the tree (level < L0)
    into complete subtrees bin-packed across 8 cores (zero cross-core edges);
    the tiny top region (level >= L0) is processed replicated on all cores
    after one AllReduce of per-top-slot child contributions.
  - Device (SPMD, one program, per-core data):
    Phase A (interleaved per 128-slot tile): x-side gate pre-activations via
      f32r GEMMs, bias folded via an appended ones-row.
    Level loop (bottom): indirect-gather parent-sorted edge-contribution
      chunks from a contiguously written DRAM buffer, segment-sum via 0/1
      S-matrix f32r matmuls into PSUM, iou GEMM, gate elementwise, f-gate
      GEMM, write [h||f*c] contribution rows.
    Cut edges (parent in top region): pre-reduced per padded top slot with an
      S-matmul, AllReduce'd across cores.
    Top levels: same pipeline, SBUF-resident, replicated on all cores.
"""

import numpy as np
from contextlib import ExitStack

N_CORES = 8
P = 128
MD = 256
TOP_CAP = 160


# ---------------------------------------------------------------- host side


def _pieces(lb, nl):
    # f32r matmuls need PSUM dst base partition 0, so every piece is processed
    # at partition base 0 (bottom: per-piece phase A; top: extended pieces).
    out = []
    s = lb
    end = lb + nl
    while s < end:
        take = min(128, end - s)
        out.append((s, take))
        s += take
    return out


def _preprocess(parent):
    parent = np.asarray(parent, dtype=np.int64)
    N = len(parent)
    level = np.zeros(N, dtype=np.int64)
    for j in range(N - 1):
        p = parent[j]
        if level[p] <= level[j]:
            level[p] = level[j] + 1
    n_levels = int(level.max()) + 1

    cnt_ge = np.zeros(n_levels + 1, dtype=np.int64)
    for l in range(n_levels - 1, -1, -1):
        cnt_ge[l] = cnt_ge[l + 1] + int((level == l).sum())
    L0 = n_levels
    for l in range(n_levels + 1):
        if cnt_ge[l] <= TOP_CAP:
            L0 = l
            break
    assert 1 <= L0 < n_levels, f"degenerate tree: L0={L0} n_levels={n_levels}"

    is_top = level >= L0
    top_ids = np.arange(N)[is_top][np.argsort(level[is_top], kind="stable")]

    # top slot layout: levels packed back-to-back, each within one 128-tile
    TNL = [int((level[top_ids] == l).sum()) for l in range(L0, n_levels)]
    TLB = []
    b = 0
    for nl in TNL:
        assert nl <= 128, "top level too large"
        if b % 128 + nl > 128:
            b = (b // 128 + 1) * 128
        TLB.append(b)
        b += nl
    TSLOTS = (b + 127) // 128 * 128
    tslot = np.full(N, -1, dtype=np.int64)
    li = 0
    for l in range(L0, n_levels):
        sel = top_ids[level[top_ids] == l]
        tslot[sel] = TLB[li] + np.arange(len(sel))
        li += 1

    # subtree partition of the bottom
    anchor = np.full(N, -1, dtype=np.int64)
    for j in range(N - 1, -1, -1):
        if is_top[j]:
            continue
        p = parent[j]
        anchor[j] = j if (p == N or is_top[p]) else anchor[p]
    roots = np.unique(anchor[anchor >= 0])
    sizes = np.zeros(N, dtype=np.int64)
    np.add.at(sizes, anchor[anchor >= 0], 1)
    order = roots[np.argsort(-sizes[roots], kind="stable")]
    load = np.zeros(N_CORES, dtype=np.int64)
    core_of_root = {}
    for r in order:
        c = int(np.argmin(load))
        core_of_root[int(r)] = c
        load[c] += sizes[r]
    core = np.full(N, -1, dtype=np.int64)
    bot = anchor >= 0
    core[bot] = [core_of_root[int(a)] for a in anchor[bot]]

    nodes_cl = [[[] for _ in range(L0)] for _ in range(N_CORES)]
    for j in np.arange(N)[bot]:
        nodes_cl[int(core[j])][int(level[j])].append(int(j))
    NL = [max(len(nodes_cl[c][l]) for c in range(N_CORES)) for l in range(L0)]

    LB = []
    b = 0
    for l in range(L0):
        LB.append(b)
        b += NL[l]
    NBOT = b
    ZIDX = NBOT
    XTOP = NBOT
    NPHA = NBOT + TSLOTS

    slot_of = np.full(N, -1, dtype=np.int64)
    node_at = [np.full(NPHA, -1, dtype=np.int64) for _ in range(N_CORES)]
    for c in range(N_CORES):
        for l in range(L0):
            for i, j in enumerate(nodes_cl[c][l]):
                slot_of[j] = LB[l] + i
                node_at[c][LB[l] + i] = j
    for j in top_ids:
        for c in range(N_CORES):
            node_at[c][XTOP + tslot[j]] = j

    meta = dict(
        N=N, L0=L0, n_levels=n_levels, level=level, parent=parent,
        NL=NL, LB=LB, NBOT=NBOT, ZIDX=ZIDX, XTOP=XTOP, NPHA=NPHA,
        TNL=TNL, TLB=TLB, TSLOTS=TSLOTS, tslot=tslot, top_ids=top_ids,
        slot_of=slot_of, node_at=node_at, core=core, is_top=is_top,
    )

    child_by_parent = [[] for _ in range(N)]
    for j in range(N - 1):
        child_by_parent[int(parent[j])].append(j)

    # ---- bottom segment chunks ----
    bot_chunks = []
    Sb_list = []
    gidx_list = []
    for l in range(1, L0):
        for (s0, cnt) in _pieces(LB[l], NL[l]):
            percore = []
            for c in range(N_CORES):
                ed = []
                for pl in range(cnt):
                    node = node_at[c][s0 + pl]
                    if node < 0:
                        continue
                    for ch in child_by_parent[node]:
                        ed.append((slot_of[ch], pl))
                percore.append(ed)
            nch = max((len(e) + P - 1) // P for e in percore)
            if nch == 0:
                continue
            for k in range(nch):
                S = np.zeros((N_CORES, P, P), dtype=np.float32)
                gi = np.full((N_CORES, P), ZIDX, dtype=np.int32)
                for c in range(N_CORES):
                    seg = percore[c][k * P:(k + 1) * P]
                    for e, (cs, pl) in enumerate(seg):
                        gi[c, e] = cs
                        S[c, e, pl] = 1.0
                Sb_list.append(S)
                gidx_list.append(gi)
            bot_chunks.append((l, s0, cnt, nch))
    TOTCH = len(Sb_list)
    S_bot = (np.stack(Sb_list, 1).reshape(N_CORES, TOTCH * P, P)
             if TOTCH else np.zeros((N_CORES, P, P), np.float32))
    gidx = (np.stack(gidx_list, 2).reshape(N_CORES, P, TOTCH)
            if TOTCH else np.full((N_CORES, P, 1), ZIDX, np.int32))
    meta["bot_chunks"] = bot_chunks
    meta["TOTCH"] = TOTCH

    # ---- cut edges -> pre-reduce per padded top slot ----
    cut_percore = [[] for _ in range(N_CORES)]
    for j in range(N - 1):
        p = int(parent[j])
        if not is_top[j] and is_top[p]:
            cut_percore[int(core[j])].append((slot_of[j], int(tslot[p])))
    CUTCH = max(1, max((len(e) + P - 1) // P for e in cut_percore))
    cidx = np.full((N_CORES, P, CUTCH), ZIDX, dtype=np.int32)
    TPP = TSLOTS // P
    S_cut = np.zeros((N_CORES, CUTCH, TPP, P, P), dtype=np.float32)
    for c in range(N_CORES):
        for e, (cs, ts) in enumerate(cut_percore[c]):
            k, r = divmod(e, P)
            cidx[c, r, k] = cs
            S_cut[c, k, ts // P, r, ts % P] = 1.0
    cut_piece_used = [bool(S_cut[:, :, tp].any()) for tp in range(TPP)]
    S_cut = S_cut.reshape(N_CORES, CUTCH * TPP * P, P)
    meta["CUTCH"] = CUTCH
    meta["TPP"] = TPP
    meta["cut_piece_used"] = cut_piece_used

    # ---- top segment chunks (global); pieces extended to tile base so every
    # PE op runs at partition base 0 (earlier levels in the tile are
    # recomputed idempotently, so S is cumulative over the tile prefix) ----
    top_chunks = []
    St_list = []
    li = 0
    for l in range(L0, n_levels):
        s0 = TLB[li]
        cnt = TNL[li]
        assert s0 % 128 + cnt <= 128
        tile_base = s0 // P * P
        cnt_ext = s0 % 128 + cnt
        # all top nodes in [tile_base, s0+cnt)
        sel = (tslot >= tile_base) & (tslot < s0 + cnt) & (tslot >= 0)
        per_chunk = {}
        has_cut = False
        for node in np.arange(len(tslot))[sel]:
            pl = int(tslot[node]) - tile_base
            for ch in child_by_parent[int(node)]:
                if is_top[ch]:
                    ts = int(tslot[ch])
                    per_chunk.setdefault(ts // P, np.zeros((P, P), np.float32))[
                        ts % P, pl] = 1.0
                else:
                    has_cut = True
        refs = []
        for ck, S in sorted(per_chunk.items()):
            refs.append((ck, len(St_list)))
            St_list.append(S)
        top_chunks.append((l, tile_base, cnt_ext, refs, has_cut))
        li += 1
    TOTTCH = len(St_list)
    S_top = (np.concatenate(St_list, 0) if TOTTCH
             else np.zeros((P, P), np.float32))
    meta["top_chunks"] = top_chunks
    meta["TOTTCH"] = TOTTCH

    data = dict(S_bot=S_bot, gidx=gidx, cidx=cidx, S_cut=S_cut, S_top=S_top)
    return meta, data


def _build_inputs(meta, data, embs, Wx, bx, Wh, bh, Wfh, bfh):
    N = meta["N"]
    NPHA = meta["NPHA"]
    IN = embs.shape[1]
    parent = meta["parent"]

    Wx_e = np.zeros((IN + 1, 768), dtype=np.float32)
    Wx_e[:IN] = Wx[:, :768]
    Wx_e[IN] = bx[:768] + bh
    WxF_e = np.zeros((IN + 1, 256), dtype=np.float32)
    WxF_e[:IN] = Wx[:, 768:1024]
    WxF_e[IN] = bx[768:1024] + bfh

    embs_pad = np.concatenate([embs, np.zeros((1, IN), np.float32)], 0)
    in_maps = []
    for c in range(N_CORES):
        na = meta["node_at"][c]
        sel = np.where(na >= 0, na, N)
        par = np.where(na >= 0, parent[np.clip(na, 0, N - 1)], N)
        par = np.minimum(par, N)
        eT = np.ones((IN + 1, NPHA), dtype=np.float32)
        eT[:IN] = embs_pad[sel].T
        pT = np.ones((IN + 1, NPHA), dtype=np.float32)
        pT[:IN] = embs_pad[par].T
        in_maps.append({
            "embsT": np.ascontiguousarray(eT),
            "embsparT": np.ascontiguousarray(pT),
            "Wx_e": Wx_e, "WxF_e": WxF_e,
            "Wh_e": np.ascontiguousarray(Wh, dtype=np.float32),
            "Wfh_e": np.ascontiguousarray(Wfh, dtype=np.float32),
            "S_bot": np.ascontiguousarray(data["S_bot"][c]),
            "gidx": np.ascontiguousarray(data["gidx"][c]),
            "cidx": np.ascontiguousarray(data["cidx"][c]),
            "S_cut": np.ascontiguousarray(data["S_cut"][c]),
            "S_top": np.ascontiguousarray(data["S_top"]),
        })
    return in_maps


# ---------------------------------------------------------------- device side


def _build_program(meta, IN, sim_no_collective=False):
    import concourse.bass as bass
    import concourse.tile as tile
    from concourse import bacc, mybir
    from concourse.masks import make_identity

    f32 = mybir.dt.float32
    f32r = mybir.dt.float32r
    i32 = mybir.dt.int32
    SIG = mybir.ActivationFunctionType.Sigmoid
    TANH = mybir.ActivationFunctionType.Tanh
    AX = bass.IndirectOffsetOnAxis

    NPHA = meta["NPHA"]
    NBOT = meta["NBOT"]
    TSLOTS = meta["TSLOTS"]
    XTOP = meta["XTOP"]
    TOTCH = meta["TOTCH"]
    TOTTCH = meta["TOTTCH"]
    CUTCH = meta["CUTCH"]
    TPP = meta["TPP"]
    K1 = IN + 1
    KT = [(k, min(P, K1 - k)) for k in range(0, K1, P)]
    NKT = len(KT)

    nc = bacc.Bacc("TRN2", target_bir_lowering=False, debug=False,
                   num_devices=N_CORES)

    embsT = nc.dram_tensor("embsT", [K1, NPHA], f32, kind="ExternalInput").ap()
    embsparT = nc.dram_tensor("embsparT", [K1, NPHA], f32, kind="ExternalInput").ap()
    Wx_e = nc.dram_tensor("Wx_e", [K1, 768], f32, kind="ExternalInput").ap()
    WxF_e = nc.dram_tensor("WxF_e", [K1, 256], f32, kind="ExternalInput").ap()
    Wh_e = nc.dram_tensor("Wh_e", [256, 768], f32, kind="ExternalInput").ap()
    Wfh_e = nc.dram_tensor("Wfh_e", [256, 256], f32, kind="ExternalInput").ap()
    S_bot = nc.dram_tensor("S_bot", [max(TOTCH, 1) * P, P], f32, kind="ExternalInput").ap()
    gidx = nc.dram_tensor("gidx", [P, max(TOTCH, 1)], i32, kind="ExternalInput").ap()
    cidx = nc.dram_tensor("cidx", [P, CUTCH], i32, kind="ExternalInput").ap()
    S_cut = nc.dram_tensor("S_cut", [CUTCH * TPP * P, P], f32, kind="ExternalInput").ap()
    S_top = nc.dram_tensor("S_top", [max(TOTTCH, 1) * P, P], f32, kind="ExternalInput").ap()

    out_h = nc.dram_tensor("out_h", [NPHA, 256], f32, kind="ExternalOutput").ap()

    contrib = nc.dram_tensor("contrib", [NBOT + P, 512], f32).ap()
    cc_in = nc.dram_tensor("cc_in", [TSLOTS, 512], f32).ap()
    cc_out = nc.dram_tensor("cc_out", [TSLOTS, 512], f32, addr_space="Shared").ap()

    with tile.TileContext(nc) as tc, ExitStack() as ctx:
        persist = ctx.enter_context(tc.tile_pool(name="persist", bufs=1))
        wpool = ctx.enter_context(tc.tile_pool(name="weights", bufs=1))
        xpool = ctx.enter_context(tc.tile_pool(name="xpre", bufs=4))
        epool = ctx.enter_context(tc.tile_pool(name="embs", bufs=3))
        spool = ctx.enter_context(tc.tile_pool(name="smat", bufs=4))
        gpool = ctx.enter_context(tc.tile_pool(name="gath", bufs=4))
        stage = ctx.enter_context(tc.tile_pool(name="stage", bufs=2))
        evac = ctx.enter_context(tc.tile_pool(name="evac", bufs=2))
        # PSUM budget (8 banks): big(2)x2 + seg(1)x2 + tt(1)x2 = 8
        pp_big = ctx.enter_context(tc.tile_pool(name="ps_big", bufs=2, space="PSUM"))
        pp_seg = ctx.enter_context(tc.tile_pool(name="ps_seg", bufs=2, space="PSUM"))
        pp_tt = ctx.enter_context(tc.tile_pool(name="ps_tt", bufs=2, space="PSUM"))

        ident = wpool.tile([P, P], f32)
        make_identity(nc, ident[:])

        wx_t = [wpool.tile([P, 768], f32r, tag=f"wx{i}", name=f"wx{i}") for i in range(NKT)]
        wxf_t = [wpool.tile([P, 256], f32r, tag=f"wxf{i}", name=f"wxf{i}") for i in range(NKT)]
        for i, (k0, kn) in enumerate(KT):
            nc.sync.dma_start(wx_t[i][:kn], Wx_e[k0:k0 + kn].bitcast(f32r))
            nc.sync.dma_start(wxf_t[i][:kn], WxF_e[k0:k0 + kn].bitcast(f32r))
        wh_t = [wpool.tile([P, 768], f32r, tag=f"wh{i}", name=f"wh{i}") for i in range(2)]
        wfh_t = [wpool.tile([P, 256], f32r, tag=f"wfh{i}", name=f"wfh{i}") for i in range(2)]
        for i in range(2):
            nc.sync.dma_start(wh_t[i][:], Wh_e[i * P:(i + 1) * P].bitcast(f32r))
            nc.sync.dma_start(wfh_t[i][:], Wfh_e[i * P:(i + 1) * P].bitcast(f32r))

        gidx_t = wpool.tile([P, max(TOTCH, 1)], i32)
        nc.sync.dma_start(gidx_t[:], gidx[:])
        cidx_t = wpool.tile([P, CUTCH], i32)
        nc.sync.dma_start(cidx_t[:], cidx[:])

        ztile = wpool.tile([P, 512], f32)
        nc.gpsimd.memset(ztile[:], 0.0)
        nc.sync.dma_start(contrib[NBOT:NBOT + P], ztile[:])

        topC = [persist.tile([P, 512], f32r, tag=f"topC{t}", name=f"topC{t}") for t in range(TPP)]
        ccR = [persist.tile([P, 512], f32, tag=f"ccR{t}", name=f"ccR{t}") for t in range(TPP)]
        for t in range(TPP):
            nc.gpsimd.memset(topC[t][:].bitcast(f32), 0.0)

        def phase_a(s0, cnt, persistent=False, pkey=None):
            """Compute xpre [cnt,768] and fxp [cnt,256] for slots [s0,s0+cnt)
            at partition base 0. Returns (xp, fx) tiles."""
            if persistent:
                xp = persist.tile([P, 768], f32, tag=f"xpt{pkey}", name=f"xpt{pkey}")
                fx = persist.tile([P, 256], f32, tag=f"fxt{pkey}", name=f"fxt{pkey}")
            else:
                xp = xpool.tile([P, 768], f32, tag="xp", name="xp")
                fx = xpool.tile([P, 256], f32, tag="fx", name="fx")
            ea = [epool.tile([P, P], f32r, tag=f"ea{i}", name=f"ea{i}") for i in range(NKT)]
            eb = [epool.tile([P, P], f32r, tag=f"eb{i}", name=f"eb{i}") for i in range(NKT)]
            for i, (k0, kn) in enumerate(KT):
                nc.sync.dma_start(ea[i][:kn, :cnt], embsT[k0:k0 + kn, s0:s0 + cnt].bitcast(f32r))
                nc.sync.dma_start(eb[i][:kn, :cnt], embsparT[k0:k0 + kn, s0:s0 + cnt].bitcast(f32r))
            big = pp_big.tile([P, 1024], f32, space="PSUM", tag="big", name="big")
            for n0, n1 in ((0, 512), (512, 768)):
                for i, (k0, kn) in enumerate(KT):
                    nc.tensor.matmul(big[:cnt, n0:n1], lhsT=ea[i][:kn, :cnt],
                                     rhs=wx_t[i][:kn, n0:n1],
                                     start=(i == 0), stop=(i == NKT - 1))
            for i, (k0, kn) in enumerate(KT):
                nc.tensor.matmul(big[:cnt, 768:1024], lhsT=eb[i][:kn, :cnt],
                                 rhs=wxf_t[i][:kn],
                                 start=(i == 0), stop=(i == NKT - 1))
            nc.vector.tensor_copy(xp[:cnt], big[:cnt, 0:768])
            nc.vector.tensor_copy(fx[:cnt], big[:cnt, 768:1024])
            return xp, fx

        def process_piece(s0, cnt, xp, fx, iou_ps, seg_ps, top_piece=None):
            """All tiles at partition base 0. iou_ps [0:cnt,0:768]=Wh@Hsum
            (None for leaves); seg_ps [0:cnt,256:512]=FCsum."""
            sl = slice(0, cnt)
            g_sb = stage.tile([P, 768], f32, tag="g", name="g_sb")
            if iou_ps is None:
                nc.scalar.activation(g_sb[sl, 0:512], xp[sl, 0:512], SIG)
                nc.scalar.activation(g_sb[sl, 512:768], xp[sl, 512:768], TANH)
            else:
                tmp = stage.tile([P, 768], f32, tag="tmp", name="tmp")
                nc.vector.tensor_add(tmp[sl, :], xp[sl, :], iou_ps[sl, 0:768])
                nc.scalar.activation(g_sb[sl, 0:512], tmp[sl, 0:512], SIG)
                nc.scalar.activation(g_sb[sl, 512:768], tmp[sl, 512:768], TANH)
            c_sb = stage.tile([P, 256], f32, tag="c", name="c_sb")
            nc.vector.tensor_mul(c_sb[sl, :], g_sb[sl, 0:256], g_sb[sl, 512:768])
            if seg_ps is not None:
                nc.vector.tensor_add(c_sb[sl, :], c_sb[sl, :], seg_ps[sl, 256:512])
            tc_sb = stage.tile([P, 256], f32, tag="tc", name="tc_sb")
            nc.scalar.activation(tc_sb[sl, :], c_sb[sl, :], TANH)
            if top_piece is None:
                ct = stage.tile([P, 512], f32, tag="ct", name="ct")
            else:
                ct = topC[top_piece]
            hv = ct[:, 0:256]
            fcv = ct[:, 256:512]
            nc.vector.tensor_mul(hv[sl, :], g_sb[sl, 256:512], tc_sb[sl, :])
            nc.sync.dma_start(out_h[s0:s0 + cnt], hv[sl, :].bitcast(f32))
            hT_ps = pp_tt.tile([P, 256], f32, space="PSUM", tag="tt", name="hT_ps")
            nc.tensor.transpose(hT_ps[0:P, 0:cnt], in_=hv[sl, 0:P].bitcast(f32),
                                identity=ident[sl, sl])
            nc.tensor.transpose(hT_ps[0:P, 128:128 + cnt], in_=hv[sl, 128:256].bitcast(f32),
                                identity=ident[sl, sl])
            hT_sb = evac.tile([P, 256], f32r, tag="hT", name="hT_sb")
            nc.vector.tensor_copy(hT_sb[:, 0:cnt], hT_ps[:, 0:cnt])
            nc.vector.tensor_copy(hT_sb[:, 128:128 + cnt], hT_ps[:, 128:128 + cnt])
            f_ps = pp_seg.tile([P, 512], f32, space="PSUM", tag="seg", name="f_ps")
            nc.tensor.matmul(f_ps[sl, 0:256], lhsT=hT_sb[:, 0:cnt], rhs=wfh_t[0][:],
                             start=True, stop=False)
            nc.tensor.matmul(f_ps[sl, 0:256], lhsT=hT_sb[:, 128:128 + cnt], rhs=wfh_t[1][:],
                             start=False, stop=True)
            f_sb = stage.tile([P, 256], f32, tag="f", name="f_sb")
            nc.vector.tensor_add(f_sb[sl, :], f_ps[sl, 0:256], fx[sl, :])
            nc.scalar.activation(f_sb[sl, :], f_sb[sl, :], SIG)
            nc.vector.tensor_mul(fcv[sl, :], f_sb[sl, :], c_sb[sl, :])
            if top_piece is None:
                nc.sync.dma_start(contrib[s0:s0 + cnt], ct[sl, :])

        def seg_to_iou(seg_ps, cnt):
            sl = slice(0, cnt)
            hs_sb = evac.tile([P, 256], f32, tag="hs", name="hs_sb")
            nc.vector.tensor_copy(hs_sb[sl, :], seg_ps[sl, 0:256])
            hsT_ps = pp_tt.tile([P, 256], f32, space="PSUM", tag="tt", name="hsT_ps")
            nc.tensor.transpose(hsT_ps[0:P, 0:cnt], in_=hs_sb[sl, 0:P],
                                identity=ident[sl, sl])
            nc.tensor.transpose(hsT_ps[0:P, 128:128 + cnt], in_=hs_sb[sl, 128:256],
                                identity=ident[sl, sl])
            hsT_sb = evac.tile([P, 256], f32r, tag="hsT", name="hsT_sb")
            nc.vector.tensor_copy(hsT_sb[:, 0:cnt], hsT_ps[:, 0:cnt])
            nc.vector.tensor_copy(hsT_sb[:, 128:128 + cnt], hsT_ps[:, 128:128 + cnt])
            iou_ps = pp_big.tile([P, 1024], f32, space="PSUM", tag="big", name="iou_ps")
            for n0, n1 in ((0, 512), (512, 768)):
                for i in range(2):
                    lt = hsT_sb[:, i * 128:i * 128 + cnt]
                    nc.tensor.matmul(iou_ps[sl, n0:n1], lhsT=lt, rhs=wh_t[i][:, n0:n1],
                                     start=(i == 0), stop=(i == 1))
            return iou_ps

        # ---- leaves, then bottom levels ----
        for (s0, cnt) in _pieces(meta["LB"][0], meta["NL"][0]):
            xp, fx = phase_a(s0, cnt)
            process_piece(s0, cnt, xp, fx, None, None)

        ci = 0
        for (l, s0, cnt, nch) in meta["bot_chunks"]:
            xp, fx = phase_a(s0, cnt)
            seg_ps = pp_seg.tile([P, 512], f32, space="PSUM", tag="seg", name="seg_ps")
            for k in range(nch):
                ch = gpool.tile([P, 512], f32r, tag="ch", name="ch")
                nc.gpsimd.indirect_dma_start(
                    out=ch[:], out_offset=None, in_=contrib[:].bitcast(f32r),
                    in_offset=AX(ap=gidx_t[:, ci:ci + 1], axis=0))
                st = spool.tile([P, P], f32r, tag="sb", name="st")
                nc.sync.dma_start(st[:], S_bot[ci * P:(ci + 1) * P].bitcast(f32r))
                nc.tensor.matmul(seg_ps[0:cnt, :], lhsT=st[:, 0:cnt], rhs=ch[:],
                                 start=(k == 0), stop=(k == nch - 1))
                ci += 1
            iou_ps = seg_to_iou(seg_ps, cnt)
            process_piece(s0, cnt, xp, fx, iou_ps, seg_ps)
        assert ci == TOTCH

        # ---- phase A for top tiles (persistent; overlaps the collective) ----
        top_xf = {}
        for t in range(TPP):
            s0 = XTOP + t * P
            cnt = min(P, NPHA - s0)
            top_xf[t] = phase_a(s0, cnt, persistent=True, pkey=t)

        # ---- cut pre-reduce + AllReduce ----
        cut_sb = []
        for k in range(CUTCH):
            ch = gpool.tile([P, 512], f32r, tag="cutch", name="cutch")
            nc.gpsimd.indirect_dma_start(
                out=ch[:], out_offset=None, in_=contrib[:].bitcast(f32r),
                in_offset=AX(ap=cidx_t[:, k:k + 1], axis=0))
            cut_sb.append(ch)
        for tp in range(TPP):
            cc_sb = evac.tile([P, 512], f32, tag="ccsb", name="cc_sb")
            if meta["cut_piece_used"][tp]:
                cc_ps = pp_seg.tile([P, 512], f32, space="PSUM", tag="seg", name="cc_ps")
                for k in range(CUTCH):
                    st = spool.tile([P, P], f32r, tag="sb", name="st")
                    off = (k * TPP + tp) * P
                    nc.sync.dma_start(st[:], S_cut[off:off + P].bitcast(f32r))
                    nc.tensor.matmul(cc_ps[:], lhsT=st[:], rhs=cut_sb[k][:],
                                     start=(k == 0), stop=(k == CUTCH - 1))
                nc.vector.tensor_copy(cc_sb[:], cc_ps[:])
            else:
                nc.gpsimd.memset(cc_sb[:], 0.0)
            nc.sync.dma_start(cc_in[tp * P:(tp + 1) * P], cc_sb[:])
        if sim_no_collective:
            cpy = evac.tile([P, 512], f32, tag="ccsb", name="cc_cpy")
            for tp in range(TPP):
                nc.sync.dma_start(cpy[:], cc_in[tp * P:(tp + 1) * P])
                nc.sync.dma_start(cc_out[tp * P:(tp + 1) * P], cpy[:])
        else:
            nc.gpsimd.collective_compute(
                "AllReduce", mybir.AluOpType.add,
                replica_groups=[list(range(N_CORES))],
                ins=[cc_in[:]], outs=[cc_out[:]],
            )
        for tp in range(TPP):
            nc.sync.dma_start(ccR[tp][:], cc_out[tp * P:(tp + 1) * P])

        # ---- top levels (pieces extended to tile base; base-0 everywhere) ----
        for (l, tile_base, cnt, refs, has_cut) in meta["top_chunks"]:
            s0 = XTOP + tile_base
            tp0 = tile_base // P
            seg_ps = pp_seg.tile([P, 512], f32, space="PSUM", tag="seg", name="seg_ps")
            for i, (ck, sidx) in enumerate(refs):
                st = spool.tile([P, P], f32r, tag="sb", name="st")
                nc.sync.dma_start(st[:], S_top[sidx * P:(sidx + 1) * P].bitcast(f32r))
                nc.tensor.matmul(seg_ps[0:cnt, :], lhsT=st[:, 0:cnt], rhs=topC[ck][:],
                                 start=(i == 0), stop=(i == len(refs) - 1))
            if has_cut:
                if refs:
                    nc.vector.tensor_add(seg_ps[0:cnt, :], seg_ps[0:cnt, :], ccR[tp0][0:cnt, :])
                else:
                    nc.vector.tensor_copy(seg_ps[0:cnt, :], ccR[tp0][0:cnt, :])
            iou_ps = seg_to_iou(seg_ps, cnt)
            xp, fx = top_xf[tp0]
            process_piece(s0, cnt, xp, fx, iou_ps, seg_ps, top_piece=tp0)

    nc.compile()
    return nc


# ---------------------------------------------------------------- entry point

_CACHE = {}


def _get_program(parent_bytes, shape):
    key = (parent_bytes, shape)
    if key not in _CACHE:
        parent = np.frombuffer(parent_bytes, dtype=np.int64)
        meta, data = _preprocess(parent)
        nc = _build_program(meta, shape[1])
        _CACHE[key] = (meta, data, nc)
    return _CACHE[key]


def kernel(embs, parent, Wx, bx, Wh, bh, Wfh, bfh):
    from concourse.bass_utils import run_bass_kernel_spmd

    embs = np.asarray(embs, np.float32)
    parent = np.asarray(parent, np.int64)
    Wx = np.asarray(Wx, np.float32)
    bx = np.asarray(bx, np.float32)
    Wh = np.asarray(Wh, np.float32)
    bh = np.asarray(bh, np.float32)
    Wfh = np.asarray(Wfh, np.float32)
    bfh = np.asarray(bfh, np.float32)

    meta, data, nc = _get_program(parent.tobytes(), embs.shape)
    in_maps = _build_inputs(meta, data, embs, Wx, bx, Wh, bh, Wfh, bfh)
    res = run_bass_kernel_spmd(nc, in_maps, list(range(N_CORES)))
    return _assemble(meta, [r["out_h"] for r in res.results])


def _assemble(meta, outs):
    N = meta["N"]
    h = np.zeros((N, MD), dtype=np.float32)
    idx = np.arange(meta["NPHA"])
    for c in range(N_CORES):
        na = meta["node_at"][c]
        m = (na >= 0) & (idx < meta["NBOT"])
        h[na[m]] = outs[c][m]
    na0 = meta["node_at"][0]
    m = (na0 >= 0) & (idx >= meta["XTOP"])
    h[na0[m]] = outs[0][m]
    return h
